# revision 2
# baseline (speedup 1.0000x reference)
"""Mixture-of-Softmaxes Trainium2 kernel (Bass/Tile, 8-core data parallel).

Reference computation (per token t, hidden h[1024]):
  prior  = sigmoid(h @ prior_w + prior_b); prior /= (prior.sum(heads) + 1e-8)
  latent = tanh(h @ latent_w + latent_b).reshape(8, 1024)
  logits = latent @ output_w + output_b                # [8, 2048]
  out    = sum_n prior[n] * softmax(logits[n])         # [2048]

Sharding: data-parallel over the 8192 tokens (B*S), 1024 tokens/core.
All params replicated. Matmul inputs in bf16, accumulation fp32.

Device layout (per core, T=1024 tokens):
  hiddenT [H, T] (host pre-transposed) so both big matmuls need no
  on-device transpose:
    phase A: latT[hd] = (latent_w[:, n*H+hd*128 : +128]).T @ hiddenT -> [128, T]
             tanh -> bf16
    phase B: logits = latT_tile.T @ output_w -> [128 tok, V] in PSUM
  softmax w/o max-subtract (logits ~ N(0, 0.63^2), exp is safe), denom via
  the ACT accum_out side-output; per-head combine is one fused DVE
  scalar_tensor_tensor: acc = (E * w_n) + acc.
"""

import os
import numpy as np
import ml_dtypes

B, S, H, NH, V = 4, 2048, 1024, 8, 2048
N_CORES = 8
T = (B * S) // N_CORES          # tokens per core
P = 128
KH = H // P                     # 8 contraction chunks
ST = 512                        # phase-A moving (token) tile
N_ST = T // ST
TT_PER_ST = ST // P
N_TT = T // P
VC = 512                        # logits free-dim chunk (one PSUM bank)
NVC = V // VC
EPS = 1e-8

_CACHE = {}


def _build(with_bias):
    import concourse.bass as bass
    import concourse.mybir as mybir
    import concourse.tile as tile
    from concourse import bacc
    from concourse.bass import ts

    f32 = mybir.dt.float32
    bf16 = mybir.dt.bfloat16

    KC = KH + (1 if with_bias else 0)   # contraction chunks incl. bias row
    HD = KH + (1 if with_bias else 0)   # logits contraction chunks
    Hx = KC * P

    nc = bacc.Bacc("TRN2", target_bir_lowering=False, debug=False)

    hT_d = nc.dram_tensor("hiddenT", [Hx, T], bf16, kind="ExternalInput")
    pw_d = nc.dram_tensor("prior_w", [Hx, NH], bf16, kind="ExternalInput")
    lw_d = nc.dram_tensor("latent_w", [Hx, NH * H], bf16, kind="ExternalInput")
    ow_d = nc.dram_tensor("output_w", [HD * P, V], bf16, kind="ExternalInput")
    out_d = nc.dram_tensor("out", [T, V], f32, kind="ExternalOutput")

    with tile.TileContext(nc) as tc:
        with (
            tc.tile_pool(name="const", bufs=1) as const,
            tc.tile_pool(name="hid", bufs=KC) as hpool,
            tc.tile_pool(name="oww", bufs=HD) as owpool,
            tc.tile_pool(name="pww", bufs=KC) as pwpool,
            tc.tile_pool(name="lww", bufs=2 * KC) as lwpool,
            tc.tile_pool(name="lat", bufs=2 * KH) as latpool,
            tc.tile_pool(name="ee", bufs=3) as epool,
            tc.tile_pool(name="acc", bufs=TT_PER_ST + 1) as accpool,
            tc.tile_pool(name="small", bufs=4 * N_TT) as spool,
            tc.tile_pool(name="ps_lat", bufs=2, space="PSUM") as ps_lat,
            tc.tile_pool(name="ps_log", bufs=1, space="PSUM") as ps_log,
            tc.tile_pool(name="ps_pri", bufs=2, space="PSUM") as ps_pri,
        ):
            # ---- resident loads -------------------------------------------
            hT = []
            for kc in range(KC):
                t = hpool.tile([P, T], bf16, tag="hT")
                nc.sync.dma_start(t[:], hT_d[ts(kc, P), :])
                hT.append(t)
            ow = []
            for hd in range(HD):
                t = owpool.tile([P, V], bf16, tag="ow")
                nc.sync.dma_start(t[:], ow_d[ts(hd, P), :])
                ow.append(t)
            pw = []
            for kc in range(KC):
                t = pwpool.tile([P, NH], bf16, tag="pw")
                nc.sync.dma_start(t[:], pw_d[ts(kc, P), :])
                pw.append(t)
            if with_bias:
                ones_t = const.tile([P, P], bf16)
                nc.vector.memset(ones_t[:], 0.0)
                nc.vector.memset(ones_t[0:1, :], 1.0)

            # ---- prior: wgt[tt] = sigmoid(h@pw) / (sum + EPS) -------------
            wgt = []
            for tt in range(N_TT):
                pr_ps = ps_pri.tile([P, NH], f32, tag="pri")
                for kc in range(KC):
                    nc.tensor.matmul(
                        pr_ps[:],
                        hT[kc][:, ts(tt, P)],
                        pw[kc][:],
                        start=(kc == 0),
                        stop=(kc == KC - 1),
                    )
                sig = spool.tile([P, NH], f32, tag="sig")
                ssum = spool.tile([P, 1], f32, tag="ssum")
                nc.scalar.activation(
                    sig[:], pr_ps[:], mybir.ActivationFunctionType.Sigmoid,
                    accum_out=ssum[:],
                )
                nc.vector.tensor_scalar_add(ssum[:], ssum[:], float(EPS))
                inv = spool.tile([P, 1], f32, tag="inv")
                nc.vector.reciprocal(inv[:], ssum[:])
                w = spool.tile([P, NH], f32, tag="wgt")
                nc.vector.tensor_scalar_mul(w[:], sig[:], inv[:])
                wgt.append(w)

            # ---- main: per 512-token supertile, per head ------------------
            for st in range(N_ST):
                acc = {}
                for n in range(NH):
                    # phase A: latT[hd] [128, ST] bf16 = tanh(lw_n.T @ hT_st)
                    lw_n = []
                    for kc in range(KC):
                        t = lwpool.tile([P, H], bf16, tag="lw")
                        nc.sync.dma_start(
                            t[:], lw_d[ts(kc, P), ts(n, H)]
                        )
                        lw_n.append(t)
                    latT = []
                    for hd in range(KH):
                        lat_ps = ps_lat.tile([P, ST], f32, tag="lat")
                        for kc in range(KC):
                            nc.tensor.matmul(
                                lat_ps[:],
                                lw_n[kc][:, ts(hd, P)],
                                hT[kc][:, ts(st, ST)],
                                start=(kc == 0),
                                stop=(kc == KC - 1),
                            )
                        lt = latpool.tile([P, ST], bf16, tag="latT")
                        nc.scalar.activation(
                            lt[:], lat_ps[:], mybir.ActivationFunctionType.Tanh
                        )
                        latT.append(lt)

                    # phase B: logits -> exp -> weighted accumulate
                    for tti in range(TT_PER_ST):
                        tt = st * TT_PER_ST + tti
                        lg_ps = ps_log.tile([P, V], f32, tag="log")
                        for hd in range(HD):
                            lhsT = (
                                latT[hd][:, ts(tti, P)]
                                if hd < KH
                                else ones_t[:]
                            )
                            for vc in range(NVC):
                                nc.tensor.matmul(
                                    lg_ps[:, ts(vc, VC)],
                                    lhsT,
                                    ow[hd][:, ts(vc, VC)],
                                    start=(hd == 0),
                                    stop=(hd == HD - 1),
                                )
                        E = epool.tile([P, V], bf16, tag="E")
                        dsm = spool.tile([P, 1], f32, tag="dsm")
                        nc.scalar.activation(
                            E[:], lg_ps[:], mybir.ActivationFunctionType.Exp,
                            accum_out=dsm[:],
                        )
                        invd = spool.tile([P, 1], f32, tag="invd")
                        nc.vector.reciprocal(invd[:], dsm[:])
                        wn = spool.tile([P, 1], f32, tag="wn")
                        nc.vector.tensor_tensor(
                            wn[:], wgt[tt][:, n:n + 1], invd[:],
                            op=mybir.AluOpType.mult,
                        )
                        if n == 0:
                            a = accpool.tile([P, V], f32, tag="acc")
                            acc[tti] = a
                            nc.vector.tensor_scalar_mul(a[:], E[:], wn[:])
                        else:
                            a = acc[tti]
                            nc.vector.scalar_tensor_tensor(
                                a[:], E[:], wn[:], a[:],
                                op0=mybir.AluOpType.mult,
                                op1=mybir.AluOpType.add,
                            )
                        if n == NH - 1:
                            nc.sync.dma_start(out_d[ts(tt, P), :], a[:])

    nc.compile()
    return nc


def _prep_inputs(hidden, prior_w, prior_b, latent_w, latent_b, output_w,
                 output_b, with_bias):
    bf16 = ml_dtypes.bfloat16
    hT = np.ascontiguousarray(
        hidden.reshape(-1, H).T.astype(bf16)
    )                                              # [H, B*S]
    pw = prior_w.astype(bf16)
    lw = latent_w.astype(bf16)
    ow = output_w.astype(bf16)
    if with_bias:
        hT = np.concatenate(
            [hT,
             np.ones((1, hT.shape[1]), bf16),
             np.zeros((P - 1, hT.shape[1]), bf16)], axis=0
        )
        pw = np.concatenate(
            [pw, prior_b.astype(bf16)[None, :], np.zeros((P - 1, NH), bf16)],
            axis=0)
        lw = np.concatenate(
            [lw, latent_b.astype(bf16)[None, :],
             np.zeros((P - 1, NH * H), bf16)], axis=0)
        ow = np.concatenate(
            [ow, output_b.astype(bf16)[None, :], np.zeros((P - 1, V), bf16)],
            axis=0)
    return hT, pw, lw, ow


def kernel(hidden, prior_w, prior_b, latent_w, latent_b, output_w, output_b,
           _profile=False):
    from concourse.bass_utils import run_bass_kernel_spmd

    with_bias = bool(
        np.any(prior_b) or np.any(latent_b) or np.any(output_b)
    )
    key = with_bias
    if key not in _CACHE:
        _CACHE[key] = _build(with_bias)
    nc = _CACHE[key]

    hT, pw, lw, ow = _prep_inputs(
        hidden, prior_w, prior_b, latent_w, latent_b, output_w, output_b,
        with_bias)

    in_maps = []
    for c in range(N_CORES):
        in_maps.append({
            "hiddenT": np.ascontiguousarray(hT[:, c * T:(c + 1) * T]),
            "prior_w": pw,
            "latent_w": lw,
            "output_w": ow,
        })

    res = run_bass_kernel_spmd(
        nc, in_maps, list(range(N_CORES)), trace=_profile
    )
    out = np.concatenate([res.results[c]["out"] for c in range(N_CORES)],
                         axis=0)
    if _profile:
        kernel.last_result = res
    return out.reshape(B, S, V)
